# revision 1
# baseline (speedup 1.0000x reference)
"""DecoderAttention Bass/Tile kernel for TRN2, batch-parallel over 8 NeuronCores.

Each core handles one batch element:
  q = enc @ Qs + Qbs ; k = enc @ Ks + Kbs ; v = nrp @ Vs + Vbs   (per head)
  scores = q k^T / sqrt(64), causal mask (-1e5), softmax
  out = (attn @ v) @ O + Ob

Layout strategy (matmuls in fp32r at full PE rate):
  - enc/nrp transposed on-device (PE transpose) to [d, s]
  - weights pre-packed host-side to [d, (h dh)]; Vs padded to [d, 16*65]
    with a ones column per head so attn@v also produces softmax row sums
  - scoresT [m, q] per head so exp output feeds attn@v without transposing
  - causal diagonal blocks masked by a GpSimd affine_select zeroing exp output
  - exp folds the 1/sqrt(d_head) scale; no max subtraction (scores are O(1),
    masked entries become exactly 0)
  - q/k projections for pair g+1 are interleaved into pair g's attention as
    PE filler work, so the tensor engine never idles long enough for the HAM
    clock gate to re-throttle it to 1.2 GHz
  - softmax division deferred: one batched reciprocal at the end, broadcast
    across partitions with one-hot selector matmuls
"""

import numpy as np

import concourse.bass as bass
import concourse.mybir as mybir
import concourse.tile as tile
from concourse import bacc
from concourse.bass_utils import run_bass_kernel_spmd
from concourse.masks import make_identity

N_HEADS, D_MODEL, D_HEAD = 16, 1024, 64
BATCH, SEQ = 8, 1024
P = 128
DCH = D_MODEL // P       # 8 contraction chunks
ST = SEQ // P            # 8 seq tiles
PAIRS = N_HEADS // 2     # 8 head pairs
VW = 65                  # v width per head incl. ones column
VTOT = N_HEADS * VW      # 1040
IGNORE = -100000.0
SCALE = 1.0 / np.sqrt(np.float32(D_HEAD))

F32 = mybir.dt.float32
F32R = mybir.dt.float32r
BF16 = mybir.dt.bfloat16
AF = mybir.ActivationFunctionType

_CACHE = {}


def _bank_splits(q0):
    # PSUM-bank-aligned (n0, nw) column splits covering [q0, SEQ)
    if q0 < 512:
        return [(q0, 512 - q0), (512, 512)]
    return [(q0, SEQ - q0)]


def _bcast_row_ap(src, n):
    # DMA access pattern replicating a [n]-element DRAM row to 128 partitions
    return bass.AP(tensor=src.tensor, offset=src.offset, ap=[[0, P], [1, n]])


def _build_program(debug=False):
    nc = bacc.Bacc("TRN2", target_bir_lowering=False, debug=False, num_devices=8)

    enc = nc.dram_tensor("enc", [SEQ, D_MODEL], F32R, kind="ExternalInput").ap()
    nrp = nc.dram_tensor("nrp", [SEQ, D_MODEL], F32R, kind="ExternalInput").ap()
    qst = nc.dram_tensor("qst", [D_MODEL, D_MODEL], F32R, kind="ExternalInput").ap()
    kst = nc.dram_tensor("kst", [D_MODEL, D_MODEL], F32R, kind="ExternalInput").ap()
    vst = nc.dram_tensor("vst", [D_MODEL, VTOT], F32R, kind="ExternalInput").ap()
    ow = nc.dram_tensor("ow", [D_MODEL, D_MODEL], F32R, kind="ExternalInput").ap()
    qb = nc.dram_tensor("qb", [D_MODEL], F32, kind="ExternalInput").ap()
    kb = nc.dram_tensor("kb", [D_MODEL], F32, kind="ExternalInput").ap()
    vb = nc.dram_tensor("vb", [VTOT], F32, kind="ExternalInput").ap()
    ob = nc.dram_tensor("ob", [D_MODEL], F32, kind="ExternalInput").ap()
    out = nc.dram_tensor("out", [SEQ, D_MODEL], F32, kind="ExternalOutput").ap()
    sums_dram = nc.dram_tensor("sums_scratch", [N_HEADS, SEQ], F32).ap()
    rcp_dram = nc.dram_tensor("rcp_scratch", [P, P], F32R).ap()
    dbg = None
    if debug:
        dbg = {
            "qt0": nc.dram_tensor("d_qt0", [P, SEQ], F32, kind="ExternalOutput").ap(),
            "kt0": nc.dram_tensor("d_kt0", [P, SEQ], F32, kind="ExternalOutput").ap(),
            "va0": nc.dram_tensor("d_va0", [P, VTOT], F32, kind="ExternalOutput").ap(),
            "zt": nc.dram_tensor("d_zt", [DCH, P, SEQ], F32, kind="ExternalOutput").ap(),
        }

    with tile.TileContext(nc) as tc:
        _kernel(tc, out, enc, nrp, qst, kst, vst, ow, qb, kb, vb, ob,
                sums_dram=sums_dram, rcp_dram=rcp_dram, dbg=dbg)
    nc.compile()
    return nc


def _kernel(tc, out, enc, nrp, qst, kst, vst, ow, qb, kb, vb, ob,
            sums_dram=None, rcp_dram=None, dbg=None):
    nc = tc.nc

    smalls = tc.alloc_tile_pool(name="smalls", bufs=1)
    identf = smalls.tile([P, P], F32, tag="identf", name="identf")
    make_identity(nc, identf)
    ident = smalls.tile([P, P], F32R, tag="ident", name="ident")
    nc.vector.tensor_copy(ident, identf)
    ident_bf = smalls.tile([P, P], BF16, tag="ident_bf", name="ident_bf")
    make_identity(nc, ident_bf)
    # M0[m, q] = IGNORE where m > q else 0 (strict causal mask, diag block)
    mask_bf = smalls.tile([P, P], BF16, tag="mask_bf", name="mask_bf")
    nc.gpsimd.memset(mask_bf, 0.0)
    nc.gpsimd.affine_select(
        out=mask_bf, in_=mask_bf,
        compare_op=mybir.AluOpType.is_ge,
        fill=IGNORE, base=0,
        pattern=[[1, P]], channel_multiplier=-1,
    )
    vb_bc = smalls.tile([P, VTOT], F32, tag="vb_bc", name="vb_bc")
    nc.sync.dma_start(out=vb_bc, in_=_bcast_row_ap(vb, VTOT))
    ob_bc = smalls.tile([P, D_MODEL], F32, tag="ob_bc", name="ob_bc")
    nc.sync.dma_start(out=ob_bc, in_=_bcast_row_ap(ob, D_MODEL))
    qb_col = smalls.tile([P, PAIRS], F32, tag="qb_col", name="qb_col")
    nc.sync.dma_start(out=qb_col, in_=qb.rearrange("(g p) -> p g", p=P))
    kb_col = smalls.tile([P, PAIRS], F32, tag="kb_col", name="kb_col")
    nc.sync.dma_start(out=kb_col, in_=kb.rearrange("(g p) -> p g", p=P))

    enc_t_pool = tc.alloc_tile_pool(name="encT", bufs=1, side="right")
    nrp_t_pool = tc.alloc_tile_pool(name="nrpT", bufs=1, side="right")
    encT = [enc_t_pool.tile([P, SEQ], F32R, tag=f"encT{c}", name=f"encT{c}") for c in range(DCH)]
    nrpT = [nrp_t_pool.tile([P, SEQ], F32R, tag=f"nrpT{c}", name=f"nrpT{c}") for c in range(DCH)]

    # ---- phase 1: transpose enc and nrp into [d, s] ----
    with tc.tile_pool(name="trin", bufs=2) as trin, \
         tc.tile_pool(name="trps", bufs=1, space="PSUM") as trps:
        for src, dst in ((enc, encT), (nrp, nrpT)):
            for tq in range(0, ST, 4):
                ptiles = [trps.tile([P, 4 * P], F32R, tag=f"tr{c}", name=f"tr{c}") for c in range(DCH)]
                for t in range(tq, tq + 4):
                    s_in = trin.tile([P, D_MODEL], F32R, tag="s_in", name="s_in")
                    nc.sync.dma_start(out=s_in, in_=src[t * P:(t + 1) * P, :])
                    for c in range(DCH):
                        nc.tensor.transpose(
                            ptiles[c][:, (t - tq) * P:(t - tq + 1) * P],
                            s_in[:, c * P:(c + 1) * P],
                            ident,
                        )
                for c in range(DCH):
                    nc.any.tensor_copy(dst[c][:, tq * P:(tq + 4) * P], ptiles[c])

    # ---- phase 2ab: q/k projections (dedicated phase, weights prefetched) ----
    qt_pool = tc.alloc_tile_pool(name="qt", bufs=1)
    kt_pool = tc.alloc_tile_pool(name="kt", bufs=1)
    qt = [qt_pool.tile([P, SEQ], F32R, tag=f"qt{g}", name=f"qt{g}") for g in range(PAIRS)]
    kt = [kt_pool.tile([P, SEQ], F32R, tag=f"kt{g}", name=f"kt{g}") for g in range(PAIRS)]
    with tc.tile_pool(name="wsb", bufs=1) as wsb, \
         tc.tile_pool(name="pproj", bufs=1, space="PSUM") as pproj:
        wqk = {}
        for pfx, wsrc in (("q", qst), ("k", kst)):
            w = [wsb.tile([P, D_MODEL], F32R, tag=f"{pfx}w{c}", name=f"{pfx}w{c}") for c in range(DCH)]
            for c in range(DCH):
                nc.scalar.dma_start(out=w[c], in_=wsrc[c * P:(c + 1) * P, :])
            wqk[pfx] = w
        for pfx, bcol, dsts in (("q", qb_col, qt), ("k", kb_col, kt)):
            w = wqk[pfx]
            for n0 in range(0, SEQ, 512):
                ptiles = [pproj.tile([P, 512], F32, tag=f"pp{g}", name=f"pp{g}") for g in range(PAIRS)]
                for c in range(DCH):
                    for g in range(PAIRS):
                        nc.tensor.matmul(
                            ptiles[g],
                            w[c][:, g * P:(g + 1) * P],
                            encT[c][:, n0:n0 + 512],
                            start=(c == 0), stop=(c == DCH - 1),
                        )
                for g in range(PAIRS):
                    nc.vector.tensor_scalar_add(
                        out=dsts[g][:, n0:n0 + 512],
                        in0=ptiles[g],
                        scalar1=bcol[:, g:g + 1],
                    )

    # ---- phase 2: v projection -> va [m, 16*65] with ones columns ----
    va_pool = tc.alloc_tile_pool(name="va", bufs=1)
    va = [va_pool.tile([P, VTOT], F32R, tag=f"va{t}", name=f"va{t}") for t in range(ST)]
    with tc.tile_pool(name="vsb", bufs=1) as vsb, \
         tc.tile_pool(name="pv", bufs=2, space="PSUM") as pv:
        vw = [vsb.tile([P, VTOT], F32R, tag=f"vw{c}", name=f"vw{c}") for c in range(DCH)]
        for c in range(DCH):
            nc.scalar.dma_start(out=vw[c], in_=vst[c * P:(c + 1) * P, :])
        for t in range(ST):
            pt = pv.tile([P, VTOT], F32, tag="pv", name="pvt")
            for c in range(DCH):
                for n0 in range(0, VTOT, 512):
                    nw = min(512, VTOT - n0)
                    nc.tensor.matmul(
                        pt[:, n0:n0 + nw],
                        nrpT[c][:, t * P:(t + 1) * P],
                        vw[c][:, n0:n0 + nw],
                        start=(c == 0), stop=(c == DCH - 1),
                    )
            # vb_bc has the per-(h,dh) bias, with 1.0 in each ones-column slot;
            # matmul wrote 0 there (vst ones-columns are zero), so add gives 1.0
            nc.vector.tensor_add(va[t], pt, vb_bc)

    nrp_t_pool.release()
    enc_t_pool.release()

    # ---- phase 3: attention, with next pair's q/k projection interleaved ----
    zt_pool = tc.alloc_tile_pool(name="zt", bufs=1)
    zt = [zt_pool.tile([P, SEQ], F32R, tag=f"zt{k}", name=f"zt{k}") for k in range(DCH)]
    osb = tc.alloc_tile_pool(name="osb", bufs=1)
    owt = [osb.tile([P, D_MODEL], F32R, tag=f"ow{k}", name=f"owt{k}") for k in range(DCH)]
    for k in range(DCH):
        nc.scalar.dma_start(out=owt[k], in_=ow[k * P:(k + 1) * P, :])

    with tc.tile_pool(name="attn", bufs=3) as apool, \
         tc.tile_pool(name="rcp", bufs=1) as rpool, \
         tc.tile_pool(name="selp", bufs=1) as selp, \
         tc.tile_pool(name="ps_s", bufs=2, space="PSUM") as spool, \
         tc.tile_pool(name="ps_z", bufs=2, space="PSUM") as zpool:
        # sel[g][j, p] = 1 where j == 2g + p // 64  (K=16 one-hot broadcast)
        sel = []
        for b in range(PAIRS):
            self_f = selp.tile([N_HEADS, P], F32, tag="self", name="self", bufs=2)
            nc.gpsimd.memset(self_f, 0.0)
            nc.gpsimd.affine_select(
                out=self_f.rearrange("j (a c) -> j a c", a=2),
                in_=self_f.rearrange("j (a c) -> j a c", a=2),
                compare_op=mybir.AluOpType.not_equal,
                fill=1.0, base=-2 * b,
                pattern=[[-1, 2], [0, D_HEAD]], channel_multiplier=1,
            )
            s_r = selp.tile([N_HEADS, P], F32R, tag=f"sel{b}", name=f"sel{b}")
            nc.vector.tensor_copy(s_r, self_f)
            sel.append(s_r)

        for h in range(N_HEADS):
            g, off = h // 2, (h % 2) * D_HEAD
            pz = zpool.tile([VW, SEQ], F32, tag="pz", name="pz")

            def av_mms(i, ae):
                q0 = i * P
                for n0, nw in _bank_splits(q0):
                    nc.tensor.matmul(
                        pz[:, n0:n0 + nw],
                        va[i][:, h * VW:(h + 1) * VW],
                        ae[:, n0:n0 + nw],
                        start=(i == 0), stop=(i == ST - 1),
                        skip_group_check=True,
                    )

            pend = None
            for i in range(ST):
                q0 = i * P
                ps = spool.tile([P, SEQ], F32, tag="ps", name="ps")
                ae = apool.tile([P, SEQ], F32R, tag="ae", name="ae")
                for n0, nw in _bank_splits(q0):
                    nc.tensor.matmul(
                        ps[:, n0:n0 + nw],
                        kt[g][off:off + D_HEAD, q0:q0 + P],
                        qt[g][off:off + D_HEAD, n0:n0 + nw],
                        start=True, stop=(n0 != q0),
                        skip_group_check=True,
                    )
                # causal diag mask: accumulate I.T @ M0
                nc.tensor.matmul(
                    ps[:, q0:q0 + P],
                    ident_bf, mask_bf,
                    start=False, stop=True,
                    skip_group_check=True,
                )
                nc.scalar.activation(
                    out=ae[:, q0:SEQ], in_=ps[:, q0:SEQ],
                    func=AF.Exp, scale=float(SCALE),
                )
                # attn@v delayed one chunk so exp latency hides behind PE work
                if pend is not None:
                    av_mms(*pend)
                pend = (i, ae)
            av_mms(*pend)
            # stash unnormalized zT and the denominator row; frees PSUM slots
            nc.vector.tensor_copy(zt[g][off:off + D_HEAD, :], pz[0:D_HEAD, :])
            srow = rpool.tile([1, SEQ], F32, tag="srow", name="srow", bufs=2)
            nc.scalar.copy(out=srow, in_=pz[D_HEAD:VW, :])
            nc.sync.dma_start(out=sums_dram[h:h + 1, :], in_=srow)

        # normalize: reciprocal over the sums reshaped to [128, 128] so the
        # FD-bound iterative divide runs across partitions (1.3us vs 6.5us),
        # then reload in [16, SEQ] layout for the broadcast matmuls
        s128 = rpool.tile([P, P], F32, tag="s128", name="s128")
        nc.sync.dma_start(out=s128, in_=sums_dram.rearrange("h (a c) -> (h a) c", c=P))
        r128 = rpool.tile([P, P], F32R, tag="r128", name="r128")
        with nc.allow_low_precision(reason="softmax denominators are O(1); fp32r rounding is fine"):
            nc.vector.reciprocal(out=r128, in_=s128)
        nc.sync.dma_start(out=rcp_dram, in_=r128)
        r16 = rpool.tile([N_HEADS, SEQ], F32R, tag="r16", name="r16")
        nc.sync.dma_start(out=r16, in_=rcp_dram.rearrange("(h a) c -> h (a c)", h=N_HEADS))
        for gg in range(PAIRS):
            pb = spool.tile([P, SEQ], F32, tag="ps", name="psb")
            for n0 in (0, 512):
                nc.tensor.matmul(
                    pb[:, n0:n0 + 512], sel[gg], r16[:, n0:n0 + 512],
                    start=True, stop=True,
                )
            nc.vector.tensor_mul(zt[gg], zt[gg], pb)

    if dbg is not None:
        nc.sync.dma_start(out=dbg["va0"], in_=va[0].bitcast(F32))
        for k in range(DCH):
            nc.sync.dma_start(out=dbg["zt"][k], in_=zt[k].bitcast(F32))

    # ---- phase 4: output projection out[s, d] = zt.T @ O + ob ----
    with tc.tile_pool(name="outsb", bufs=3) as outsb, \
         tc.tile_pool(name="po", bufs=2, space="PSUM") as po:
        for t in range(ST):
            pt = po.tile([P, D_MODEL], F32, tag="po", name="pot")
            for k in range(DCH):
                for n0 in range(0, D_MODEL, 512):
                    nc.tensor.matmul(
                        pt[:, n0:n0 + 512],
                        zt[k][:, t * P:(t + 1) * P],
                        owt[k][:, n0:n0 + 512],
                        start=(k == 0), stop=(k == DCH - 1),
                    )
            ot = outsb.tile([P, D_MODEL], F32, tag="ot", name="ot")
            nc.vector.tensor_add(ot, pt, ob_bc)
            nc.sync.dma_start(out=out[t * P:(t + 1) * P, :], in_=ot)

    for pool in (osb, zt_pool, va_pool, kt_pool, qt_pool, smalls):
        pool.release()


def _get_program():
    if "nc" not in _CACHE:
        _CACHE["nc"] = _build_program()
    return _CACHE["nc"]


def _pack_weights(Qs, Qbs, Ks, Kbs, Vs, Vbs, O, Ob):
    f = np.float32
    qst = np.ascontiguousarray(np.transpose(np.asarray(Qs, f), (1, 0, 2)).reshape(D_MODEL, D_MODEL))
    kst = np.ascontiguousarray(np.transpose(np.asarray(Ks, f), (1, 0, 2)).reshape(D_MODEL, D_MODEL))
    vst = np.zeros((D_MODEL, VTOT), f)
    vb = np.zeros((VTOT,), f)
    Vs = np.asarray(Vs, f)
    Vbs = np.asarray(Vbs, f)
    for h in range(N_HEADS):
        vst[:, h * VW:h * VW + D_HEAD] = Vs[h]
        vb[h * VW:h * VW + D_HEAD] = Vbs[h]
        vb[h * VW + D_HEAD] = 1.0
    ow = np.ascontiguousarray(np.asarray(O, f).reshape(D_MODEL, D_MODEL))
    qbf = np.ascontiguousarray(np.asarray(Qbs, f).reshape(D_MODEL))
    kbf = np.ascontiguousarray(np.asarray(Kbs, f).reshape(D_MODEL))
    obf = np.ascontiguousarray(np.asarray(Ob, f).reshape(D_MODEL))
    return qst, kst, vst, ow, qbf, kbf, vb, obf


def kernel(normalized_resid_pre, encoder_output, Qs, Qbs, Ks, Kbs, Vs, Vbs, O, Ob,
           _trace=False, _trace_kwargs=None):
    nc = _get_program()
    qst, kst, vst, ow, qbf, kbf, vb, obf = _pack_weights(Qs, Qbs, Ks, Kbs, Vs, Vbs, O, Ob)
    enc = np.asarray(encoder_output, np.float32)
    nrp = np.asarray(normalized_resid_pre, np.float32)
    in_maps = []
    for b in range(BATCH):
        in_maps.append({
            "enc": np.ascontiguousarray(enc[b]),
            "nrp": np.ascontiguousarray(nrp[b]),
            "qst": qst, "kst": kst, "vst": vst, "ow": ow,
            "qb": qbf, "kb": kbf, "vb": vb, "ob": obf,
        })
    res = run_bass_kernel_spmd(
        nc, in_maps, list(range(BATCH)),
        trace=_trace, **(_trace_kwargs or {}),
    )
    out = np.stack([res.results[b]["out"] for b in range(BATCH)], axis=0)
    if _trace:
        _CACHE["last_results"] = res
    return out

